# revision 23
# baseline (speedup 1.0000x reference)
"""Trainium2 Bass kernel for nn_ExcEmbedding (ragged caption/image cosine sims).

Sharding: caption batch AND image batch both split across 8 cores (32 each).
Per core:
  1. img rows (1152, 1024): leaky-relu + square split across ACT (Prelu) and
     DVE, bf16; indicator-matmul reductions (bf16, 16KB DMA descriptors)
     -> s1, s2 PSUM -> v = s1 * rsqrt(s2 * R^2)  (bf16).
  2. v^T via PE transposes; my_vv = [v^T | (v^T)^2] (128, 512) bf16; remote
     SBUF broadcast to all 8 cores. The NRT kernel barrier is waited on in
     its own early critical so the mesh runs concurrent with the img phase;
     the broadcast critical only generates descs + triggers + waits arrival.
  3. cap rows (2048, 1024): masked sum + full sum-of-squares -> cv bf16.
     (cap input DMAs are issued AFTER the broadcast critical in program
     order so the critical's entry snapshot does not wait on them.)
  4. SE gate in transposed layout (bf16 matmuls).
  5. sims^T(c, i) via 24 bf16 matmuls (vg, q2 first, then num) + fused
     epilogue (reciprocal_approx_fast, per-partition sqrt scales).
  6. host assembles sims[:, cols_j] = simsT_j.T
"""

import numpy as np
import ml_dtypes

import concourse.bass as bass
import concourse.bacc as bacc
import concourse.mybir as mybir
import concourse.tile as tile
from concourse.bass_utils import run_bass_kernel_spmd

F32 = mybir.dt.float32
BF16 = mybir.dt.bfloat16

NCORES = 8
B = 256
R = 36
T = 64
D = 1024
DSQ = 128
M = B // NCORES          # 32 local captions / images per core
KD = D // 128            # 8 d-blocks
SEG = KD * M             # 256 cols per rank segment of vv: [k, c]

# (row_start, nrows) DMA tiles; nrows/128 rows per partition, contiguous.
IMG_TILES = [(0, 512), (512, 512), (1024, 128)]
CAP_TILES = [(0, 512), (512, 512), (1024, 512), (1536, 256), (1792, 128), (1920, 128)]
NI_CH = sum(nr // 128 for _, nr in IMG_TILES)    # 9 (tile, r) chunks
NC_CH = sum(nr // 128 for _, nr in CAP_TILES)    # 16

# chunks (global index among 25) that run fully on DVE (leaky+square);
# all other chunks: leaky on ACT (Prelu), square on DVE.
DVE_ONLY = {0, 7, 12, 19}


def build_program(beta: float, skip_input: bool = False):
    nc = bacc.Bacc("TRN2", target_bir_lowering=False, debug=False,
                   num_devices=NCORES)

    img_rows = nc.dram_tensor("img_rows", [M * R, D], F32, kind="ExternalInput")
    cap_rows = nc.dram_tensor("cap_rows", [M * T, D], F32, kind="ExternalInput")
    # indicators, partition-major per 128-row chunk: [128, chunk, M]
    ei_t = nc.dram_tensor("ei_t", [128, NI_CH * M], BF16, kind="ExternalInput")
    # cap: [128, chunk, 2M]: cols 0:M masked (t < lens), M:2M unmasked
    ec2_t = nc.dram_tensor("ec2_t", [128, NC_CH * 2 * M], BF16, kind="ExternalInput")
    # W_sq^T layout: [128, KD*128]: wsq[p, 128k+j] = W_sq[128k+p, j]
    w_sq = nc.dram_tensor("w_sq", [128, D], BF16, kind="ExternalInput")
    w_ex = nc.dram_tensor("w_ex", [DSQ, D], BF16, kind="ExternalInput")
    b_sq_t = nc.dram_tensor("b_sq_t", [DSQ, 1], F32, kind="ExternalInput")
    b_ex_t = nc.dram_tensor("b_ex_t", [128, KD * M], F32, kind="ExternalInput")
    lens2 = nc.dram_tensor("lens2", [M, 1], F32, kind="ExternalInput")
    idn128 = nc.dram_tensor("idn128", [128, 128], BF16, kind="ExternalInput")
    simst_out = nc.dram_tensor("simst_out", [M, B], F32, kind="ExternalOutput")

    rsem = nc.alloc_semaphore(name="rsem")
    lsem = nc.alloc_semaphore(name="lsem")
    psem = nc.alloc_semaphore(name="psem")

    with tile.TileContext(nc) as tc:
        with (
            tc.tile_pool(name="consts", bufs=1) as consts,
            tc.tile_pool(name="xin", bufs=4) as xin,
            tc.tile_pool(name="ypool", bufs=2) as ypool,
            tc.tile_pool(name="y2pool", bufs=2) as y2pool,
            tc.tile_pool(name="ep", bufs=1) as ep,
            tc.tile_pool(name="smalls", bufs=1) as smalls,
            tc.tile_pool(name="pimg", bufs=1, space="PSUM") as pimg,
            tc.tile_pool(name="pcap", bufs=1, space="PSUM") as pcap,
        ):
            mult = mybir.AluOpType.mult
            amax = mybir.AluOpType.max
            AF = mybir.ActivationFunctionType

            # Keep the NRT prelude AllGather in the NEFF (co-dispatch) but do
            # not wait on it: data safety comes from rsem counting + no
            # entry clears (sems are cleared at the END of the kernel, so
            # in-flight increments can never be erased).
            with tc.tile_critical():
                rank = nc.gpsimd.partition_id()

            # ---- consts (bf16 from host) on the ACT HWDGE queue ----
            idn_sb = consts.tile([128, 128], BF16)
            nc.scalar.dma_start(idn_sb[:], idn128[:])
            bsq_sb = consts.tile([128, 1], F32)
            nc.scalar.dma_start(bsq_sb[:], b_sq_t[:])
            bex_sb = consts.tile([128, KD * M], F32)
            nc.scalar.dma_start(bex_sb[:], b_ex_t[:])
            lens2_sb = consts.tile([M, 1], F32)
            nc.scalar.dma_start(lens2_sb[:], lens2[:])
            ei_sb = consts.tile([128, NI_CH, M], BF16)
            nc.scalar.dma_start(ei_sb[:], ei_t[:].rearrange("p (t c) -> p t c", t=NI_CH))
            ec_sb = consts.tile([128, NC_CH, 2 * M], BF16)
            nc.scalar.dma_start(ec_sb[:], ec2_t[:].rearrange("p (t c) -> p t c", t=NC_CH))
            wsq_sb = consts.tile([128, D], BF16)
            nc.scalar.dma_start(wsq_sb[:], w_sq[:])
            wex_sb = consts.tile([128, D], BF16)
            nc.scalar.dma_start(wex_sb[:], w_ex[:])

            def leaky_square(y, y2, x, cs, gch):
                # y = leaky_relu(x, 0.1), y2 = y*y on the chunk cols cs
                if gch in DVE_ONLY:
                    nc.vector.scalar_tensor_tensor(y[:, cs], x[:, cs], 0.1,
                                                   x[:, cs], op0=mult, op1=amax)
                else:
                    nc.scalar.activation(y[:, cs], x[:, cs], AF.Prelu, alpha=0.1)
                nc.vector.tensor_mul(y2[:, cs], y[:, cs], y[:, cs])

            # ---- img input stream (sync HWDGE queue, 16KB descriptors) ----
            hp_img = tc.high_priority()
            hp_img.__enter__()
            img_x = []
            for (r0, nr) in IMG_TILES:
                x = xin.tile([128, 4096], F32, tag="xin", name=f"xi{r0}")
                rpp = nr // 128
                if not skip_input:
                    nc.sync.dma_start(
                        x[:, 0:rpp * D],
                        img_rows[r0:r0 + nr, :].rearrange("(p r) d -> p (r d)", p=128))
                else:
                    nc.sync.dma_start(x[:, 0:D], img_rows[r0:r0 + 128, :].rearrange("(p r) d -> p (r d)", p=128))
                img_x.append((x, rpp))

            # ---- img phase: s1 = sum_r y, s2 = sum_r y^2 per (img, d) ----
            s1a = pimg.tile([M, 512], F32, tag="sa", name="s1a")
            s1b = pimg.tile([M, 512], F32, tag="sb", name="s1b")
            s2a = pimg.tile([M, 512], F32, tag="sc", name="s2a")
            s2b = pimg.tile([M, 512], F32, tag="sd", name="s2b")
            s1h = [s1a, s1b]
            s2h = [s2a, s2b]
            ch = 0
            for (x, rpp) in img_x:
                y = ypool.tile([128, 4096], BF16, tag="y", name="y")
                y2 = y2pool.tile([128, 4096], BF16, tag="y2", name="y2")
                for r in range(rpp):
                    cs = slice(D * r, D * (r + 1))
                    leaky_square(y, y2, x, cs, ch)
                    for h in range(2):
                        ms = slice(D * r + 512 * h, D * r + 512 * (h + 1))
                        nc.tensor.matmul(s1h[h][:], ei_sb[:, ch, :], y[:, ms],
                                         start=(ch == 0), stop=(ch == NI_CH - 1),
                                         skip_group_check=True)
                        nc.tensor.matmul(s2h[h][:], ei_sb[:, ch, :], y2[:, ms],
                                         start=(ch == 0), stop=(ch == NI_CH - 1),
                                         skip_group_check=True)
                    ch += 1

            # ---- img epilogue: v = s1 * rsqrt(s2 * R^2)  (bf16) ----
            sqv = ep.tile([M, D], F32, tag="sq", name="sqv")
            nc.scalar.activation(sqv[:, 0:512], s2a[:], AF.Sqrt, scale=float(R * R))
            nc.scalar.activation(sqv[:, 512:D], s2b[:], AF.Sqrt, scale=float(R * R))
            rcpv = ep.tile([M, D], F32, tag="rcp", name="rcpv")
            nc.vector.reciprocal_approx_fast(rcpv[:], sqv[:])
            v_bf = smalls.tile([M, D], BF16, name="v_bf")
            nc.vector.tensor_mul(v_bf[:, 0:512], rcpv[:, 0:512], s1a[:])
            nc.vector.tensor_mul(v_bf[:, 512:D], rcpv[:, 512:D], s1b[:])

            # ---- v^T via PE transposes; my_vv = [v^T | (v^T)^2] bf16 ----
            vt_ps = pimg.tile([128, KD * M], BF16, tag="sa", name="vt_ps")
            for k in range(KD):
                nc.tensor.transpose(vt_ps[:, M * k:M * (k + 1)],
                                    v_bf[0:M, 128 * k:128 * (k + 1)],
                                    idn_sb[0:M, 0:M])
            my_vv = smalls.tile([128, SEG], BF16, name="my_vv")
            nc.scalar.copy(my_vv[:], vt_ps[:])

            # ---- broadcast my_vv into vv on all 8 cores; wait for all ----
            vv = smalls.tile([128, NCORES * SEG], BF16, name="vv")
            vv2 = smalls.tile([128, NCORES * SEG], BF16, name="vv2")
            with tc.tile_critical():
                nc.gpsimd.remote_dma_broadcast(
                    vv[:, bass.ds(rank * SEG, SEG)], my_vv[:],
                    remote_sem=rsem, local_sem=lsem,
                    rdests=[(0, j) for j in range(NCORES)],
                ).then_inc(psem, 1)
                nc.gpsimd.wait_ge(psem, 1)
                nc.gpsimd.bir_kernel_barrier_wait([list(range(NCORES))])
                nc.gpsimd.trigger_dma(count=1)
                nc.gpsimd.wait_ge(rsem, NCORES * 2)
            hp_img.__exit__(None, None, None)
            # V^2 on DVE (idle when the data lands; wait-hint keeps the
            # scheduler from hoisting it into the cap stream)
            with tc.tile_wait_until(0.080):
                nc.vector.tensor_mul(vv2[:], vv[:], vv[:])
            vv4 = vv[:].rearrange("p (g k c) -> p g k c", g=NCORES, k=KD)
            vv24 = vv2[:].rearrange("p (g k c) -> p g k c", g=NCORES, k=KD)

            # ---- cap input stream (issued after the critical in program
            # order so the critical's entry snapshot is img-phase only) ----
            cap_x = []
            for (r0, nr) in CAP_TILES:
                x = xin.tile([128, 4096], F32, tag="xin", name=f"xc{r0}")
                rpp = nr // 128
                if not skip_input:
                    nc.sync.dma_start(
                        x[:, 0:rpp * D],
                        cap_rows[r0:r0 + nr, :].rearrange("(p r) d -> p (r d)", p=128))
                else:
                    nc.sync.dma_start(x[:, 0:D], cap_rows[r0:r0 + 128, :].rearrange("(p r) d -> p (r d)", p=128))
                cap_x.append((x, rpp))

            # ---- cap phase: m1 = masked sum y, s2c = full sum y^2 ----
            m1a = pcap.tile([M, 512], F32, tag="ca", name="m1a")
            m1b = pcap.tile([M, 512], F32, tag="cb", name="m1b")
            s2ca = pcap.tile([M, 512], F32, tag="cc", name="s2ca")
            s2cb = pcap.tile([M, 512], F32, tag="cd", name="s2cb")
            m1h = [m1a, m1b]
            s2ch = [s2ca, s2cb]
            ch = 0
            for (x, rpp) in cap_x:
                y = ypool.tile([128, 4096], BF16, tag="y", name="y")
                y2 = y2pool.tile([128, 4096], BF16, tag="y2", name="y2")
                for r in range(rpp):
                    cs = slice(D * r, D * (r + 1))
                    leaky_square(y, y2, x, cs, NI_CH + ch)
                    for h in range(2):
                        ms = slice(D * r + 512 * h, D * r + 512 * (h + 1))
                        nc.tensor.matmul(m1h[h][:], ec_sb[:, ch, 0:M], y[:, ms],
                                         start=(ch == 0), stop=(ch == NC_CH - 1),
                                         skip_group_check=True)
                        nc.tensor.matmul(s2ch[h][:], ec_sb[:, ch, M:2 * M], y2[:, ms],
                                         start=(ch == 0), stop=(ch == NC_CH - 1),
                                         skip_group_check=True)
                    ch += 1

            # ---- cap epilogue: cv = m1 * rsqrt(s2c * lens^2)  (bf16) ----
            sqc = ep.tile([M, D], F32, tag="sq", name="sqc")
            nc.scalar.activation(sqc[:, 0:512], s2ca[:], AF.Sqrt, scale=lens2_sb[:])
            nc.scalar.activation(sqc[:, 512:D], s2cb[:], AF.Sqrt, scale=lens2_sb[:])
            rcpc = ep.tile([M, D], F32, tag="rcp", name="rcpc")
            nc.vector.reciprocal_approx_fast(rcpc[:], sqc[:])
            cv_bf = smalls.tile([M, D], BF16, name="cv_bf")
            nc.vector.tensor_mul(cv_bf[:, 0:512], rcpc[:, 0:512], m1a[:])
            nc.vector.tensor_mul(cv_bf[:, 512:D], rcpc[:, 512:D], m1b[:])

            # row stats on DVE (ACT stays free for the gate chain)
            cv2t = ep.tile([M, D], BF16, tag="cv2t", name="cv2t")
            nc.vector.tensor_mul(cv2t[:], cv_bf[:], cv_bf[:])
            nrm2 = smalls.tile([M, 1], F32, name="nrm2")
            nc.vector.reduce_sum(nrm2[:], cv2t[:], axis=mybir.AxisListType.X)
            cvsum = smalls.tile([M, 1], F32, name="cvsum")
            nc.vector.reduce_sum(cvsum[:], cv_bf[:], axis=mybir.AxisListType.X)
            bnum = smalls.tile([M, 1], F32, name="bnum")
            nc.vector.tensor_scalar_mul(bnum[:], cvsum[:], beta)

            # pre-swap ACT to the sigmoid table while ACT is idle
            sdum = smalls.tile([M, 1], F32, name="sdum")
            nc.scalar.activation(sdum[:], lens2_sb[:], AF.Sigmoid)

            # ---- cv^T via PE transposes -> cvt (128, KD*M) bf16 ----
            cvt_ps = pcap.tile([128, KD * M], BF16, tag="cb", name="cvt_ps")
            for k in range(KD):
                nc.tensor.transpose(cvt_ps[:, M * k:M * (k + 1)],
                                    cv_bf[0:M, 128 * k:128 * (k + 1)],
                                    idn_sb[0:M, 0:M])
            cvt = smalls.tile([128, KD * M], BF16, name="cvt")
            nc.vector.tensor_copy(cvt[:], cvt_ps[:])

            # ---- gate in transposed layout ----
            ht_ps = pimg.tile([128, M], F32, tag="sa", name="ht_ps")
            for k in range(KD):
                nc.tensor.matmul(ht_ps[:], wsq_sb[:, 128 * k:128 * (k + 1)],
                                 cvt[:, M * k:M * (k + 1)],
                                 start=(k == 0), stop=(k == KD - 1),
                                 skip_group_check=True)
            ht = smalls.tile([128, M], BF16, name="ht")
            nc.scalar.activation(ht[:], ht_ps[:], AF.Relu, bias=bsq_sb[:], scale=1.0)

            gps = pimg.tile([128, KD * M], F32, tag="sb", name="gps")
            for k in range(KD):
                sk = slice(M * k, M * (k + 1))
                nc.tensor.matmul(gps[:, sk], wex_sb[:, 128 * k:128 * (k + 1)],
                                 ht[:], skip_group_check=True)
            gz = smalls.tile([128, KD * M], F32, name="gz")
            nc.vector.tensor_add(gz[:], gps[:], bex_sb[:])
            gt = smalls.tile([128, KD * M], BF16, name="gt")
            nc.scalar.activation(gt[:], gz[:], AF.Sigmoid)
            tdum = smalls.tile([M, 1], F32, name="tdum")
            nc.scalar.sqrt(tdum[:], lens2_sb[:])
            g2t = smalls.tile([128, KD * M], BF16, name="g2t")
            nc.vector.tensor_mul(g2t[:], gt[:], gt[:])
            at = smalls.tile([128, KD * M], BF16, name="at")
            nc.vector.tensor_mul(at[:], gt[:], cvt[:])

            # ---- final matmuls: vg/q2 first (feed the sqrt chain), num last --
            wait_finals = tc.tile_wait_until(0.060)
            wait_finals.__enter__()
            vg_ps = pimg.tile([M, B], F32, tag="sd", name="vg_ps")
            q2_ps = pcap.tile([M, B], F32, tag="ca", name="q2_ps")
            num_ps = pimg.tile([M, B], F32, tag="sc", name="num_ps")
            for k in range(KD):
                sk = slice(M * k, M * (k + 1))
                nc.tensor.matmul(vg_ps[:], gt[:, sk], vv4[:, :, k, :],
                                 start=(k == 0), stop=(k == KD - 1),
                                 skip_group_check=True)
            for k in range(KD):
                sk = slice(M * k, M * (k + 1))
                nc.tensor.matmul(num_ps[:], at[:, sk], vv4[:, :, k, :],
                                 start=(k == 0), stop=(k == KD - 1),
                                 skip_group_check=True)
            for k in range(KD):
                sk = slice(M * k, M * (k + 1))
                nc.tensor.matmul(q2_ps[:], g2t[:, sk], vv24[:, :, k, :],
                                 start=(k == 0), stop=(k == KD - 1),
                                 skip_group_check=True)

            wait_finals.__exit__(None, None, None)

            # ---- end-of-run sem clears: safe here (finals consumed vv; the
            # next run's earliest remote increment trails its sender's img
            # phase by ~35us, far after these clears) ----
            with tc.tile_critical():
                nc.gpsimd.sem_clear(rsem)
                nc.gpsimd.sem_clear(lsem)
                nc.gpsimd.sem_clear(psem)

            # ---- epilogue: sims^T = (num + b*cvsum) / sqrt((q2+2b*vg+b^2*D)*nrm2)
            beta2d = smalls.tile([M, 1], F32, name="beta2d")
            nc.vector.memset(beta2d[:], beta * beta * D)
            qt = smalls.tile([M, B], F32, name="qt")
            nc.scalar.activation(qt[:], vg_ps[:], AF.Identity,
                                 bias=beta2d[:], scale=2.0 * beta)
            qs = smalls.tile([M, B], F32, name="qs")
            nc.vector.tensor_add(qs[:], qt[:], q2_ps[:])
            sq = smalls.tile([M, B], F32, name="sq")
            nc.scalar.activation(sq[:], qs[:], AF.Sqrt, scale=nrm2[:])
            rq = smalls.tile([M, B], F32, name="rq")
            nc.vector.reciprocal_approx_fast(rq[:], sq[:])
            nt = smalls.tile([M, B], F32, name="nt")
            nc.scalar.activation(nt[:], num_ps[:], AF.Identity, bias=bnum[:], scale=1.0)
            simst = smalls.tile([M, B], F32, name="simst")
            nc.vector.tensor_mul(simst[:], nt[:], rq[:])
            nc.scalar.dma_start(simst_out[:], simst[:])

    nc.compile()
    return nc


_PROG_CACHE: dict = {}


def get_program(beta: float):
    key = beta
    if key not in _PROG_CACHE:
        _PROG_CACHE[key] = build_program(beta)
    return _PROG_CACHE[key]


def make_in_maps(img_embed, cap_embed, lens, W_sq, b_sq, W_ex, b_ex):
    img_embed = np.ascontiguousarray(img_embed, dtype=np.float32)
    cap_embed = np.ascontiguousarray(cap_embed, dtype=np.float32)
    lens_i = np.asarray(lens).astype(np.int64)

    # W_sq^T layout: wsq[p, 128k+j] = W_sq[128k+p, j]
    w_sq_np = np.ascontiguousarray(
        np.asarray(W_sq, dtype=np.float32).reshape(KD, 128, DSQ)
        .transpose(1, 0, 2).reshape(128, KD * DSQ)).astype(ml_dtypes.bfloat16)
    w_ex_np = np.ascontiguousarray(W_ex, dtype=np.float32).astype(ml_dtypes.bfloat16)
    b_sq_np = np.ascontiguousarray(np.asarray(b_sq, dtype=np.float32).reshape(DSQ, 1))
    b_ex_np = np.ascontiguousarray(np.repeat(
        np.asarray(b_ex, dtype=np.float32).reshape(KD, 128).T[:, :, None],
        M, axis=2).reshape(128, KD * M))

    # img indicator per 128-row chunk: partition p of chunk (t, r) holds row
    # r0 + rpp*p + r; ei[p, ch, c] = 1 if that row's image == c
    ei_np = np.zeros((128, NI_CH, M), dtype=np.float32)
    ch = 0
    for (r0, nr) in IMG_TILES:
        rpp = nr // 128
        for r in range(rpp):
            rows = r0 + rpp * np.arange(128) + r
            ei_np[np.arange(128), ch, rows // R] = 1.0
            ch += 1
    ei_t_np = np.ascontiguousarray(
        ei_np.reshape(128, NI_CH * M)).astype(ml_dtypes.bfloat16)

    in_maps = []
    for j in range(NCORES):
        sl = slice(M * j, M * (j + 1))
        lens_local = lens_i[sl]
        ec_np = np.zeros((128, NC_CH, 2 * M), dtype=np.float32)
        ch = 0
        for (r0, nr) in CAP_TILES:
            rpp = nr // 128
            for r in range(rpp):
                rows = r0 + rpp * np.arange(128) + r
                caps = rows // T
                tidx = rows % T
                ec_np[np.arange(128), ch, M + caps] = 1.0
                keep = tidx < lens_local[caps]
                ec_np[np.arange(128)[keep], ch, caps[keep]] = 1.0
                ch += 1
        ec2_t_np = np.ascontiguousarray(
            ec_np.reshape(128, NC_CH * 2 * M)).astype(ml_dtypes.bfloat16)
        lens2_np = (lens_local.astype(np.float32) ** 2).reshape(M, 1)

        in_maps.append({
            "img_rows": np.ascontiguousarray(img_embed[sl].reshape(M * R, D)),
            "cap_rows": np.ascontiguousarray(cap_embed[sl].reshape(M * T, D)),
            "ei_t": ei_t_np,
            "ec2_t": ec2_t_np,
            "w_sq": w_sq_np,
            "w_ex": w_ex_np,
            "b_sq_t": b_sq_np,
            "b_ex_t": b_ex_np,
            "lens2": lens2_np,
            "idn128": np.eye(128, dtype=ml_dtypes.bfloat16),
        })
    return in_maps


LAST_RESULT = None


def kernel(img_embed, cap_embed, lens, W_sq, b_sq, W_ex, b_ex, beta, beta1):
    global LAST_RESULT
    beta_f = float(np.asarray(beta).reshape(-1)[0])
    nc = get_program(beta_f)
    in_maps = make_in_maps(img_embed, cap_embed, lens, W_sq, b_sq, W_ex, b_ex)
    res = run_bass_kernel_spmd(nc, in_maps, core_ids=list(range(NCORES)))
    LAST_RESULT = res
    sims = np.empty((B, B), dtype=np.float32)
    for j in range(NCORES):
        sims[:, M * j:M * (j + 1)] = res.results[j]["simst_out"].T
    return sims
